# revision 24
# baseline (speedup 1.0000x reference)
"""Multi-head attention (b=2, n=2048, d_model=1024, H=16, d_k=d_v=64) on 8
Trainium2 NeuronCores.

Sharding: 8 cores = 2 (batch) x 4 (head groups of 4 heads).  Each core
computes, for its batch ib and head group g (heads 4g..4g+3):

    kT = Wk_g @ x_k^T            [256, 2048]   (d' on partitions, bf16)
    V  = x_v @ Wv_g^T            [2048, 256]   (keys on partitions, bf16)
    qT = Wq_g @ x_q^T            [256, 2048]
    per q-chunk of 512, per key-block kt of 128, per head pair:
       S^T = K Q^T  via two row-tiled (K=64) concurrent matmuls -> PSUM f32
       at  = exp(S^T/8)          one ACTIVATE per pair  [128, 1024] -> bf16
       O^T += V_h^T A^T   via two col-tiled (M=64) concurrent matmuls
       Z   += 1^T A^T     via four col-tiled (M=1) matmuls (denominators)
    1/Z broadcast across partitions via a DRAM roundtrip (DRE replicate),
    normalize O^T, out-projection Y^T = Wo_g @ O_cat^T  [1024, 2048] f32.

Host sums the 4 per-group partial Y^T per batch and adds bo.

All matmuls run in bf16 (inputs quantized on host); accumulation is fp32 in
PSUM.  Softmax skips the max-subtraction: scores*scale are ~N(0,1) so exp
never overflows.  The scalar engine (exp: 16.8M elements/core at 1 elem/
lane/cycle ~= 145us) is the critical path; matmuls, DMA and vector work are
scheduled to hide underneath it.  DMAs are issued in first-use order so the
first exp fires ~15us in.
"""

import numpy as np
from contextlib import ExitStack

import ml_dtypes

import concourse.bass as bass
import concourse.mybir as mybir
import concourse.tile as tile
from concourse import bacc
from concourse.bass_utils import run_bass_kernel_spmd

F32 = mybir.dt.float32
BF16 = mybir.dt.bfloat16
F16 = mybir.dt.float16
EXP = mybir.ActivationFunctionType.Exp
ADD = mybir.AluOpType.add
MULT = mybir.AluOpType.mult

D_MODEL = 1024
H = 16
DK = 64
B = 2
N = 2048           # nq = nk
G = 4              # head groups (cores per batch)
HG = H // G        # heads per group = 4
DG = HG * DK       # 256 group dims
KT = 8             # D_MODEL / 128 contraction tiles
NKT = N // 128     # 16 key blocks in attention
QC = 512           # attention q-chunk
NCH = N // QC      # 4 chunks
P = 128

_PROGRAM = None


def _build_program():
    nc = bacc.Bacc("TRN2", target_bir_lowering=False, debug=False, num_devices=8)

    # host-pretiled inputs; every DMA partition line is contiguous
    xqT = nc.dram_tensor("xqT", [P, NCH, KT, QC], BF16, kind="ExternalInput").ap()
    xkT = nc.dram_tensor("xkT", [P, NCH, KT, QC], BF16, kind="ExternalInput").ap()
    xvT = nc.dram_tensor("xvT", [P, NKT, KT, P], BF16, kind="ExternalInput").ap()
    wqT = nc.dram_tensor("wqT", [P, KT, DG], BF16, kind="ExternalInput").ap()
    wkT = nc.dram_tensor("wkT", [P, KT, DG], BF16, kind="ExternalInput").ap()
    wvT = nc.dram_tensor("wvT", [P, KT, DG], BF16, kind="ExternalInput").ap()
    woT = nc.dram_tensor("woT", [P, 2, D_MODEL], BF16, kind="ExternalInput").ap()
    bq_d = nc.dram_tensor("bq_s", [P, 2], F32, kind="ExternalInput").ap()
    bk_d = nc.dram_tensor("bk_s", [P, 2], F32, kind="ExternalInput").ap()
    ones_c_d = nc.dram_tensor("ones_c", [P, 1], BF16, kind="ExternalInput").ap()
    zeros_d = nc.dram_tensor("zeros_w", [P, P], BF16, kind="ExternalInput").ap()
    yT_d = nc.dram_tensor("yT", [D_MODEL, N], BF16, kind="ExternalOutput").ap()
    # dram staging for softmax denominators (internal DRAM tiles fail to load
    # under the axon PJRT path, so an ExternalOutput buffer instead)
    z_st = nc.dram_tensor("z_st", [NCH, 4 * QC], F32, kind="ExternalOutput").ap()

    with tile.TileContext(nc) as tc:
        with ExitStack() as ctx:
            const = ctx.enter_context(tc.tile_pool(name="const", bufs=1))
            xin = ctx.enter_context(tc.tile_pool(name="xin", bufs=1))
            pers = ctx.enter_context(tc.tile_pool(name="pers", bufs=1))
            atp = ctx.enter_context(tc.tile_pool(name="atp", bufs=4))
            osb = ctx.enter_context(tc.tile_pool(name="osb", bufs=2))
            ysb = ctx.enter_context(tc.tile_pool(name="ysb", bufs=3))
            rzs = ctx.enter_context(tc.tile_pool(name="rzs", bufs=4))
            # PSUM: spool 2x[128,1024]f32 = 4 banks, av 2x1, z 1, y 1 = 8
            spool = ctx.enter_context(tc.tile_pool(name="spool", bufs=2, space="PSUM"))
            avp = ctx.enter_context(tc.tile_pool(name="avp", bufs=2, space="PSUM"))
            zp = ctx.enter_context(tc.tile_pool(name="zp", bufs=1, space="PSUM"))
            yp = ctx.enter_context(tc.tile_pool(name="yp", bufs=1, space="PSUM"))

            # ---- constants (scalar HWDGE queue; x loads go on sync) ----
            bq_sb = const.tile([P, 2], F32, tag="bq")
            nc.scalar.dma_start(bq_sb[:], bq_d)

            wk_sb = const.tile([P, KT, DG], BF16, tag="wk")
            wq_sb = const.tile([P, KT, DG], BF16, tag="wq")
            wv_sb = const.tile([P, KT, DG], BF16, tag="wv")
            wo_sb = const.tile([P, 2, D_MODEL], BF16, tag="wo")
            nc.scalar.dma_start(wq_sb[:], wqT)
            # touch exp so its ACT table set loads during warmup
            dum = const.tile([1, 2], F32, tag="dum")
            nc.scalar.activation(dum[:], bq_sb[0:1, :], EXP, scale=0.0)
            nc.scalar.dma_start(wk_sb[:], wkT)
            bk_sb = const.tile([P, 2], F32, tag="bk")
            ones_c = const.tile([P, 1], BF16, tag="onc")
            zeros_w = const.tile([P, P], BF16, tag="zw")
            nc.scalar.dma_start(bk_sb[:], bk_d)
            nc.scalar.dma_start(zeros_w[:], zeros_d)
            nc.scalar.dma_start(ones_c[:], ones_c_d)
            nc.scalar.dma_start(wv_sb[:], wvT)
            nc.scalar.dma_start(wo_sb[:], woT)

            # ---- x loads (sync HWDGE queue) in first-use order ----
            xk_sb = xin.tile([P, NCH, KT, QC], BF16, tag="xk")
            xv_sb = xin.tile([P, NKT, KT, P], BF16, tag="xv")
            xq_sb = xin.tile([P, NCH, KT, QC], BF16, tag="xq")
            nc.sync.dma_start(xk_sb[:, 0, :, 0:P], xkT[:, 0, :, 0:P])
            nc.sync.dma_start(xq_sb[:, 0], xqT[:, 0])
            nc.sync.dma_start(xk_sb[:, 0, :, P:QC], xkT[:, 0, :, P:QC])
            nc.sync.dma_start(xv_sb[:, 0:4], xvT[:, 0:4])
            nc.sync.dma_start(xk_sb[:, 1], xkT[:, 1])
            nc.sync.dma_start(xv_sb[:, 4:8], xvT[:, 4:8])
            nc.sync.dma_start(xk_sb[:, 2], xkT[:, 2])
            nc.sync.dma_start(xv_sb[:, 8:12], xvT[:, 8:12])
            nc.sync.dma_start(xk_sb[:, 3], xkT[:, 3])
            nc.sync.dma_start(xv_sb[:, 12:16], xvT[:, 12:16])
            nc.sync.dma_start(xq_sb[:, 1], xqT[:, 1])
            nc.sync.dma_start(xq_sb[:, 2], xqT[:, 2])
            nc.sync.dma_start(xq_sb[:, 3], xqT[:, 3])

            # ---- persistent activations ----
            kt_sb = pers.tile([P, 2, N], BF16, tag="kt")     # K^T, d' on part
            v_sb = pers.tile([P, NKT, HG, DK], BF16, tag="v")  # V, keys on part
            qt_sb = pers.tile([P, 2, N], BF16, tag="qt")     # Q^T

            YZ = [(yp, "y"), (zp, "z")]   # both 1-bank pools (warmup/tail only)
            YO = [(yp, "y")]              # in-chunk work must not touch zp

            def k_proj(c, pools, half=None):
                for j in ((0, 1) if half is None else (half,)):
                    pool, tg = pools[j % len(pools)]
                    ps = pool.tile([P, QC], F32, tag=tg, name=f"kps_{c}_{j}")
                    for k in range(KT):
                        nc.tensor.matmul(
                            ps[:], wk_sb[:, k, j * P:(j + 1) * P], xk_sb[:, c, k, :],
                            start=(k == 0), stop=(k == KT - 1))
                    nc.vector.tensor_tensor(
                        kt_sb[:, j, c * QC:(c + 1) * QC], ps[:],
                        bk_sb[:, j, None].to_broadcast((P, QC)), ADD)

            qps_pend = {}

            def q_proj(c, pools, half=None, ks=None):
                k0, k1 = ks if ks is not None else (0, KT)
                for j in ((0, 1) if half is None else (half,)):
                    if (c, j) in qps_pend:
                        ps = qps_pend.pop((c, j))
                    else:
                        pool, tg = pools[j % len(pools)]
                        ps = pool.tile([P, QC], F32, tag=tg, name=f"qps_{c}_{j}")
                    for k in range(k0, k1):
                        nc.tensor.matmul(
                            ps[:], wq_sb[:, k, j * P:(j + 1) * P], xq_sb[:, c, k, :],
                            start=(k == 0), stop=(k == KT - 1))
                    if k1 < KT:
                        qps_pend[(c, j)] = ps
                        return
                    nc.vector.tensor_tensor(
                        qt_sb[:, j, c * QC:(c + 1) * QC], ps[:],
                        bq_sb[:, j, None].to_broadcast((P, QC)), ADD)

            def v_proj(nt, pools):
                # note: bv is folded into the host-side gather (Wo @ bv),
                # since softmax rows sum to 1: att @ (V + bv) = att @ V + bv
                pool, tg = pools[nt % len(pools)]
                ps = pool.tile([P, QC], F32, tag=tg, name=f"vps_{nt}")
                for k in range(KT):
                    nc.tensor.matmul(ps[:, 0:DG], xv_sb[:, nt, k, :], wv_sb[:, k, :],
                                     start=(k == 0), stop=(k == KT - 1))
                nc.vector.tensor_copy(
                    v_sb[:, nt], ps[:, 0:DG].rearrange("p (h d) -> p h d", h=HG))

            def y_tile(c, m, pools):
                # out-projection m-tile of chunk c: Y^T[m*128:+128, cQC:+QC]
                pool, tg = pools[m % len(pools)]
                yps = pool.tile([P, QC], F32, tag=tg, name=f"yps_{c}_{m}")
                o_c = o_tiles[c]
                for j in range(2):
                    nc.tensor.matmul(
                        yps[:], wo_sb[:, j, m * P:(m + 1) * P], o_c[:, j, :],
                        start=(j == 0), stop=(j == 1))
                y_sb = ysb.tile([P, QC], BF16, tag="ysb", name=f"ysb_{c}_{m}")
                nc.vector.tensor_copy(y_sb[:], yps[:])
                eng = nc.scalar if (c == NCH - 1 and m % 2 == 1) else nc.sync
                eng.dma_start(
                    yT_d[m * P:(m + 1) * P, c * QC:(c + 1) * QC], y_sb[:])

            o_tiles = {}
            avs = {}


            def s_exp(c, kt):
                # S^T for one key block: 2 pairs x 2 row-tiled matmuls + exp
                ats = []
                for pair in range(2):
                    sps = spool.tile([P, 2 * QC], F32, tag="s",
                                     name=f"sps_{c}_{kt}_{pair}")
                    for hp in range(2):
                        p0 = 64 * hp
                        nc.tensor.matmul(
                            sps[:, hp * QC:(hp + 1) * QC],
                            kt_sb[p0:p0 + 64, pair, kt * P:(kt + 1) * P],
                            qt_sb[p0:p0 + 64, pair, c * QC:(c + 1) * QC],
                            start=True, stop=True,
                            tile_position=(p0, 0))
                    at = atp.tile([P, 2 * QC], BF16, tag="at",
                                  name=f"at_{c}_{kt}_{pair}")
                    nc.scalar.activation(at[:], sps[:], EXP, scale=0.125)
                    ats.append(at)
                return ats

            def k_proj0_part(j, q0, q1, pool, tg):
                # K projection of chunk 0 restricted to key columns [q0, q1)
                w = q1 - q0
                ps = pool.tile([P, QC], F32, tag=tg, name=f"kp0_{j}_{q0}")
                for k in range(KT):
                    nc.tensor.matmul(
                        ps[:, 0:w], wk_sb[:, k, j * P:(j + 1) * P],
                        xk_sb[:, 0, k, q0:q1],
                        start=(k == 0), stop=(k == KT - 1))
                nc.vector.tensor_tensor(
                    kt_sb[:, j, q0:q1], ps[:, 0:w],
                    bk_sb[:, j, None].to_broadcast((P, w)), ADD)

            # ---- warmup: K block 0 (128 keys), Q chunk 0, first S/exp,
            # then the rest of K chunk 0 and V block 0 ----
            k_proj0_part(0, 0, P, yp, "y")
            q_proj(0, YZ, half=0)
            k_proj0_part(1, 0, P, zp, "z")
            q_proj(0, YZ, half=1)
            ats_chunk = s_exp(0, 0)
            k_proj0_part(0, P, QC, yp, "y")
            k_proj0_part(1, P, QC, zp, "z")
            v_proj(0, YZ)

            for c in range(NCH):
                av0 = avp.tile([P, QC], F32, tag="av", name=f"av0_{c}")
                av1 = avp.tile([P, QC], F32, tag="av", name=f"av1_{c}")
                zps = zp.tile([P, QC], F32, tag="z", name=f"zps_{c}")
                avs[c] = (av0, av1)

                # S/exp runs one key-block ahead of AV so accumulator-reuse
                # waits (normalize of chunk c-1) never starve the ACT.
                ats_next = ats_chunk

                for kt in range(NKT):
                    ats = ats_next

                    if kt + 1 < NKT:
                        ats_next = s_exp(c, kt + 1)

                    # -- interleaved projection / output work on the PE;
                    # emitted after the next S/exp so it never gates ACT --
                    if c == 0:
                        if kt in (1, 2):
                            k_proj(1, YO, half=kt - 1)
                        if kt in (5, 6):
                            k_proj(2, YO, half=kt - 5)
                        if kt in (9, 10):
                            k_proj(3, YO, half=kt - 9)
                        if kt < NKT - 1:
                            v_proj(kt + 1, YO)
                    if c > 0 and 4 <= kt <= 11:
                        y_tile(c - 1, kt - 4, YO)
                    if c < NCH - 1 and 11 <= kt <= 14:
                        j, part = (kt - 11) // 2, (kt - 11) % 2
                        q_proj(c + 1, YO, half=j,
                               ks=(part * 4, 4 + part * 4))

                    if kt == 0:
                        # zero-matmuls set has_written across each whole bank
                        # so the col-tiled groups below can accumulate
                        rhs0 = xk_sb[:, 0, 0, :]
                        nc.tensor.matmul(av0[:], zeros_w[:], rhs0,
                                         start=True, stop=False)
                        nc.tensor.matmul(av1[:], zeros_w[:], rhs0,
                                         start=True, stop=False)
                        nc.tensor.matmul(zps[:], zeros_w[:], rhs0,
                                         start=True, stop=False)

                    last = kt == NKT - 1
                    if last:
                        # Z first so the denominator chain starts earlier
                        for h in range(4):
                            nc.tensor.matmul(
                                zps[32 * h:32 * h + 1, :],
                                ones_c[:],
                                ats[h // 2][:, (h % 2) * QC:(h % 2 + 1) * QC],
                                start=False, stop=(h == 3),
                                tile_position=(0, 32 * h))
                    for pair in range(2):
                        at = ats[pair]
                        av = avs[c][pair]
                        for hp in range(2):
                            h = 2 * pair + hp
                            nc.tensor.matmul(
                                av[64 * hp:64 * hp + 64, :],
                                v_sb[:, kt, h, :], at[:, hp * QC:(hp + 1) * QC],
                                start=False, stop=(last and hp == 1),
                                tile_position=(0, 64 * hp))
                    if not last:
                        for h in range(4):
                            nc.tensor.matmul(
                                zps[32 * h:32 * h + 1, :],
                                ones_c[:],
                                ats[h // 2][:, (h % 2) * QC:(h % 2 + 1) * QC],
                                start=False, stop=False,
                                tile_position=(0, 32 * h))

                # next chunk's first S/exp goes ahead of the normalize chain
                if c + 1 < NCH:
                    ats_chunk = s_exp(c + 1, 0)

                # -- free the av banks fast: unnormalized copy to f16 SBUF
                # (next chunk's zero-matmuls reuse the banks ~1.5us later,
                # instead of waiting for the whole 1/Z broadcast chain) --
                zr = rzs.tile([1, 4 * QC], F32, tag="zr", name=f"zr_{c}")
                for h in range(4):
                    nc.vector.tensor_copy(zr[0:1, h * QC:(h + 1) * QC],
                                          zps[32 * h:32 * h + 1, :])
                oun = osb.tile([P, 2, QC], F16, tag="oun", name=f"oun_{c}")
                for pair in range(2):
                    nc.vector.tensor_copy(oun[:, pair, :], avs[c][pair][:])

                # -- softmax denominators: stage via DRAM to broadcast --
                z_dram = z_st[c:c + 1, :]
                nc.sync.dma_start(z_dram, zr[:])
                z_v = z_dram.rearrange("a (h q) -> (a h) q", h=4)
                o_sb = osb.tile([P, 2, QC], BF16, tag="o", name=f"o_{c}")
                o_tiles[c] = o_sb
                for pair in range(2):
                    # after the last exp the scalar queue is free: use it to
                    # halve the tail's broadcast latency on the final chunk
                    eng = nc.scalar if (c == NCH - 1 and pair == 1) else nc.sync
                    zb = rzs.tile([P, QC], F32, tag="zb", name=f"zb_{c}_{pair}")
                    for hp in range(2):
                        h = 2 * pair + hp
                        eng.dma_start(
                            zb[64 * hp:64 * hp + 64, :],
                            z_v[h, None, :].to_broadcast((64, QC)))
                    rzb = rzs.tile([P, QC], F32, tag="rzb", name=f"rzb_{c}_{pair}")
                    nc.vector.reciprocal_approx_fast(rzb[:], zb[:])
                    nc.vector.tensor_tensor(
                        o_sb[:, pair, :], oun[:, pair, :], rzb[:], MULT)

            # tail: out-projection for the last chunk
            for m in range(8):
                y_tile(NCH - 1, m, YZ)

    nc.compile()
    return nc


def get_program():
    global _PROGRAM
    if _PROGRAM is None:
        _PROGRAM = _build_program()
    return _PROGRAM


BF = ml_dtypes.bfloat16


def _tile_xT(x, nchunk, width):
    # x [n, 1024] -> x^T tiled [128 p, nchunk, 8 k, width] bf16
    xt = np.ascontiguousarray(x.T)                      # [1024, n]
    return np.ascontiguousarray(
        xt.reshape(KT, P, nchunk, width).transpose(1, 2, 0, 3).astype(BF))


def _tile_w(w_rows):
    # w_rows [256, 1024] (= W[g-slice]) -> W^T tiled [128 p, 8 k, 256] bf16
    return np.ascontiguousarray(
        w_rows.T.reshape(KT, P, DG).transpose(1, 0, 2).astype(BF))


def make_in_maps(queries, keys, values, Wq, bq, Wk, bk, Wv, bv, Wo, bo):
    """Build per-core input dicts. Core c handles batch c//4, head group c%4."""
    f32 = np.float32
    xT = {}
    for ib in range(B):
        xT[ib] = (
            _tile_xT(np.asarray(queries[ib], f32), NCH, QC),
            _tile_xT(np.asarray(keys[ib], f32), NCH, QC),
            _tile_xT(np.asarray(values[ib], f32), NKT, P),
        )
    ones_c = np.ones((P, 1), BF)
    zeros_w = np.zeros((P, P), BF)
    in_maps = []
    for core in range(8):
        ib, g = core // G, core % G
        sl = slice(g * DG, (g + 1) * DG)
        in_maps.append({
            "xqT": xT[ib][0], "xkT": xT[ib][1], "xvT": xT[ib][2],
            "wqT": _tile_w(Wq[sl, :]),
            "wkT": _tile_w(Wk[sl, :]),
            "wvT": _tile_w(Wv[sl, :]),
            "woT": np.ascontiguousarray(
                Wo[:, sl].T.reshape(2, P, D_MODEL).transpose(1, 0, 2).astype(BF)),
            "bq_s": np.ascontiguousarray(np.asarray(bq[sl], f32).reshape(2, P).T),
            "bk_s": np.ascontiguousarray(np.asarray(bk[sl], f32).reshape(2, P).T),
            "ones_c": ones_c,
            "zeros_w": zeros_w,
        })
    return in_maps


def gather_output(results, bias):
    out = np.zeros((B, N, D_MODEL), np.float32)
    for core in range(8):
        out[core // G] += np.asarray(results[core]["yT"], np.float32).T
    out += bias[None, None, :]
    return out


def _run(inputs, trace=False, **spmd_kwargs):
    nc = get_program()
    in_maps = make_in_maps(**inputs)
    res = run_bass_kernel_spmd(nc, in_maps, core_ids=list(range(8)),
                               trace=trace, **spmd_kwargs)
    # bv commutes through the softmax (rows sum to 1) and the linear
    # out-projection, so it lands on the host as Wo @ bv; bo is host-side.
    bias = (np.asarray(inputs["Wo"], np.float64) @ np.asarray(inputs["bv"], np.float64)
            + np.asarray(inputs["bo"], np.float64)).astype(np.float32)
    return gather_output(res.results, bias), res


def kernel(**inputs) -> np.ndarray:
    out, _ = _run(inputs, trace=False)
    return out


# revision 26
# speedup vs baseline: 1.1760x; 1.1760x over previous
"""Multi-head attention (b=2, n=2048, d_model=1024, H=16, d_k=d_v=64) on 8
Trainium2 NeuronCores.

Sharding: 8 cores = 2 (batch) x 4 (head groups of 4 heads).  Each core
computes, for its batch ib and head group g (heads 4g..4g+3):

    kT = Wk_g @ x_k^T            [256, 2048]   (d' on partitions, bf16)
    V  = x_v @ Wv_g^T            [2048, 256]   (keys on partitions, bf16)
    qT = Wq_g @ x_q^T            [256, 2048]
    per q-chunk of 512, per key-block kt of 128, per head pair:
       S^T = K Q^T  via two row-tiled (K=64) concurrent matmuls -> PSUM f32
       at  = exp(S^T/8)          one ACTIVATE per pair  [128, 1024] -> bf16
       O^T += V_h^T A^T   via two col-tiled (M=64) concurrent matmuls
       Z   += 1^T A^T     via four col-tiled (M=1) matmuls (denominators)
    1/Z broadcast across partitions via a DRAM roundtrip (DRE replicate),
    normalize O^T, out-projection Y^T = Wo_g @ O_cat^T  [1024, 2048] f32.

Host sums the 4 per-group partial Y^T per batch and adds bo.

All matmuls run in bf16 (inputs quantized on host); accumulation is fp32 in
PSUM.  Softmax skips the max-subtraction: scores*scale are ~N(0,1) so exp
never overflows.  The scalar engine (exp: 16.8M elements/core at 1 elem/
lane/cycle ~= 145us) is the critical path; matmuls, DMA and vector work are
scheduled to hide underneath it.  DMAs are issued in first-use order so the
first exp fires ~15us in.
"""

import numpy as np
from contextlib import ExitStack

import ml_dtypes

import concourse.bass as bass
import concourse.mybir as mybir
import concourse.tile as tile
from concourse import bacc
from concourse.bass_utils import run_bass_kernel_spmd

F32 = mybir.dt.float32
BF16 = mybir.dt.bfloat16
F16 = mybir.dt.float16
EXP = mybir.ActivationFunctionType.Exp
ADD = mybir.AluOpType.add
MULT = mybir.AluOpType.mult

D_MODEL = 1024
H = 16
DK = 64
B = 2
N = 2048           # nq = nk
G = 4              # head groups (cores per batch)
HG = H // G        # heads per group = 4
DG = HG * DK       # 256 group dims
KT = 8             # D_MODEL / 128 contraction tiles
NKT = N // 128     # 16 key blocks in attention
QC = 512           # attention q-chunk
NCH = N // QC      # 4 chunks
P = 128

_PROGRAM = None


def _build_program():
    nc = bacc.Bacc("TRN2", target_bir_lowering=False, debug=False, num_devices=8)

    # host-pretiled inputs; every DMA partition line is contiguous
    xqT = nc.dram_tensor("xqT", [P, NCH, KT, QC], BF16, kind="ExternalInput").ap()
    xkT = nc.dram_tensor("xkT", [P, NCH, KT, QC], BF16, kind="ExternalInput").ap()
    xvT = nc.dram_tensor("xvT", [P, NKT, KT, P], BF16, kind="ExternalInput").ap()
    wqT = nc.dram_tensor("wqT", [P, KT, DG], BF16, kind="ExternalInput").ap()
    wkT = nc.dram_tensor("wkT", [P, KT, DG], BF16, kind="ExternalInput").ap()
    wvT = nc.dram_tensor("wvT", [P, KT, DG], BF16, kind="ExternalInput").ap()
    woT = nc.dram_tensor("woT", [P, 2, D_MODEL], BF16, kind="ExternalInput").ap()
    bq_d = nc.dram_tensor("bq_s", [P, 2], F32, kind="ExternalInput").ap()
    bk_d = nc.dram_tensor("bk_s", [P, 2], F32, kind="ExternalInput").ap()
    ones_c_d = nc.dram_tensor("ones_c", [P, 1], BF16, kind="ExternalInput").ap()
    zeros_d = nc.dram_tensor("zeros_w", [P, P], BF16, kind="ExternalInput").ap()
    yT_d = nc.dram_tensor("yT", [D_MODEL, N], BF16, kind="ExternalOutput").ap()
    # dram staging for softmax denominators (internal DRAM tiles fail to load
    # under the axon PJRT path, so an ExternalOutput buffer instead)
    z_st = nc.dram_tensor("z_st", [NCH, 4 * QC], F32, kind="ExternalOutput").ap()

    with tile.TileContext(nc) as tc:
        with ExitStack() as ctx:
            const = ctx.enter_context(tc.tile_pool(name="const", bufs=1))
            xin = ctx.enter_context(tc.tile_pool(name="xin", bufs=1))
            pers = ctx.enter_context(tc.tile_pool(name="pers", bufs=1))
            atp = ctx.enter_context(tc.tile_pool(name="atp", bufs=4))
            osb = ctx.enter_context(tc.tile_pool(name="osb", bufs=2))
            ysb = ctx.enter_context(tc.tile_pool(name="ysb", bufs=3))
            rzs = ctx.enter_context(tc.tile_pool(name="rzs", bufs=4))
            # PSUM: spool 2x[128,1024]f32 = 4 banks, av 2x1, z 1, y 1 = 8
            spool = ctx.enter_context(tc.tile_pool(name="spool", bufs=2, space="PSUM"))
            avp = ctx.enter_context(tc.tile_pool(name="avp", bufs=2, space="PSUM"))
            zp = ctx.enter_context(tc.tile_pool(name="zp", bufs=1, space="PSUM"))
            yp = ctx.enter_context(tc.tile_pool(name="yp", bufs=1, space="PSUM"))

            # ---- constants (scalar HWDGE queue; x loads go on sync) ----
            bq_sb = const.tile([P, 2], F32, tag="bq")
            nc.scalar.dma_start(bq_sb[:], bq_d)

            wk_sb = const.tile([P, KT, DG], BF16, tag="wk")
            wq_sb = const.tile([P, KT, DG], BF16, tag="wq")
            wv_sb = const.tile([P, KT, DG], BF16, tag="wv")
            wo_sb = const.tile([P, 2, D_MODEL], BF16, tag="wo")
            nc.scalar.dma_start(wq_sb[:], wqT)
            # touch exp so its ACT table set loads during warmup
            dum = const.tile([1, 2], F32, tag="dum")
            nc.scalar.activation(dum[:], bq_sb[0:1, :], EXP, scale=0.0)
            nc.scalar.dma_start(wk_sb[:], wkT)
            bk_sb = const.tile([P, 2], F32, tag="bk")
            ones_c = const.tile([P, 1], BF16, tag="onc")
            zeros_w = const.tile([P, P], BF16, tag="zw")
            nc.scalar.dma_start(bk_sb[:], bk_d)
            nc.scalar.dma_start(zeros_w[:], zeros_d)
            nc.scalar.dma_start(ones_c[:], ones_c_d)
            nc.scalar.dma_start(wv_sb[:], wvT)
            nc.scalar.dma_start(wo_sb[:], woT)

            # ---- x loads (sync HWDGE queue) in first-use order ----
            xk_sb = xin.tile([P, NCH, KT, QC], BF16, tag="xk")
            xv_sb = xin.tile([P, NKT, KT, P], BF16, tag="xv")
            xq_sb = xin.tile([P, NCH, KT, QC], BF16, tag="xq")
            nc.sync.dma_start(xk_sb[:, 0, :, 0:P], xkT[:, 0, :, 0:P])
            nc.sync.dma_start(xq_sb[:, 0], xqT[:, 0])
            nc.sync.dma_start(xk_sb[:, 0, :, P:QC], xkT[:, 0, :, P:QC])
            nc.sync.dma_start(xv_sb[:, 0:4], xvT[:, 0:4])
            nc.sync.dma_start(xk_sb[:, 1], xkT[:, 1])
            nc.sync.dma_start(xv_sb[:, 4:8], xvT[:, 4:8])
            nc.sync.dma_start(xk_sb[:, 2], xkT[:, 2])
            nc.sync.dma_start(xv_sb[:, 8:12], xvT[:, 8:12])
            nc.sync.dma_start(xk_sb[:, 3], xkT[:, 3])
            nc.sync.dma_start(xv_sb[:, 12:16], xvT[:, 12:16])
            nc.sync.dma_start(xq_sb[:, 1], xqT[:, 1])
            nc.sync.dma_start(xq_sb[:, 2], xqT[:, 2])
            nc.sync.dma_start(xq_sb[:, 3], xqT[:, 3])

            # ---- persistent activations ----
            kt_sb = pers.tile([P, 2, N], BF16, tag="kt")     # K^T, d' on part
            v_sb = pers.tile([P, NKT, HG, DK], BF16, tag="v")  # V, keys on part
            qt_sb = pers.tile([P, 2, N], BF16, tag="qt")     # Q^T

            YZ = [(yp, "y"), (zp, "z")]   # both 1-bank pools (warmup/tail only)
            YO = [(yp, "y")]              # in-chunk work must not touch zp

            def k_proj(c, pools, half=None):
                for j in ((0, 1) if half is None else (half,)):
                    pool, tg = pools[j % len(pools)]
                    ps = pool.tile([P, QC], F32, tag=tg, name=f"kps_{c}_{j}")
                    for k in range(KT):
                        nc.tensor.matmul(
                            ps[:], wk_sb[:, k, j * P:(j + 1) * P], xk_sb[:, c, k, :],
                            start=(k == 0), stop=(k == KT - 1))
                    nc.vector.tensor_tensor(
                        kt_sb[:, j, c * QC:(c + 1) * QC], ps[:],
                        bk_sb[:, j, None].to_broadcast((P, QC)), ADD)

            qps_pend = {}

            def q_proj(c, pools, half=None, ks=None):
                k0, k1 = ks if ks is not None else (0, KT)
                for j in ((0, 1) if half is None else (half,)):
                    if (c, j) in qps_pend:
                        ps = qps_pend.pop((c, j))
                    else:
                        pool, tg = pools[j % len(pools)]
                        ps = pool.tile([P, QC], F32, tag=tg, name=f"qps_{c}_{j}")
                    for k in range(k0, k1):
                        nc.tensor.matmul(
                            ps[:], wq_sb[:, k, j * P:(j + 1) * P], xq_sb[:, c, k, :],
                            start=(k == 0), stop=(k == KT - 1))
                    if k1 < KT:
                        qps_pend[(c, j)] = ps
                        return
                    nc.vector.tensor_tensor(
                        qt_sb[:, j, c * QC:(c + 1) * QC], ps[:],
                        bq_sb[:, j, None].to_broadcast((P, QC)), ADD)

            def v_proj(nt, pools):
                # note: bv is folded into the host-side gather (Wo @ bv),
                # since softmax rows sum to 1: att @ (V + bv) = att @ V + bv
                pool, tg = pools[nt % len(pools)]
                ps = pool.tile([P, QC], F32, tag=tg, name=f"vps_{nt}")
                for k in range(KT):
                    nc.tensor.matmul(ps[:, 0:DG], xv_sb[:, nt, k, :], wv_sb[:, k, :],
                                     start=(k == 0), stop=(k == KT - 1))
                nc.vector.tensor_copy(
                    v_sb[:, nt], ps[:, 0:DG].rearrange("p (h d) -> p h d", h=HG))

            def y_tile(c, m, pools):
                # out-projection m-tile of chunk c: Y^T[m*128:+128, cQC:+QC]
                pool, tg = pools[m % len(pools)]
                yps = pool.tile([P, QC], F32, tag=tg, name=f"yps_{c}_{m}")
                o_c = o_tiles[c]
                for j in range(2):
                    nc.tensor.matmul(
                        yps[:], wo_sb[:, j, m * P:(m + 1) * P], o_c[:, j, :],
                        start=(j == 0), stop=(j == 1))
                y_sb = ysb.tile([P, QC], BF16, tag="ysb", name=f"ysb_{c}_{m}")
                nc.vector.tensor_copy(y_sb[:], yps[:])
                eng = nc.scalar if (c == NCH - 1 and m % 2 == 1) else nc.sync
                eng.dma_start(
                    yT_d[m * P:(m + 1) * P, c * QC:(c + 1) * QC], y_sb[:])

            o_tiles = {}
            avs = {}


            def s_exp(c, kt):
                # S^T for one key block: 2 pairs x 2 row-tiled matmuls + exp
                ats = []
                for pair in range(2):
                    sps = spool.tile([P, 2 * QC], F32, tag="s",
                                     name=f"sps_{c}_{kt}_{pair}")
                    for hp in range(2):
                        p0 = 64 * hp
                        nc.tensor.matmul(
                            sps[:, hp * QC:(hp + 1) * QC],
                            kt_sb[p0:p0 + 64, pair, kt * P:(kt + 1) * P],
                            qt_sb[p0:p0 + 64, pair, c * QC:(c + 1) * QC],
                            start=True, stop=True,
                            tile_position=(p0, 0))
                    at = atp.tile([P, 2 * QC], BF16, tag="at",
                                  name=f"at_{c}_{kt}_{pair}")
                    nc.scalar.activation(at[:], sps[:], EXP, scale=0.125)
                    ats.append(at)
                return ats

            def k_proj0_part(j, q0, q1, pool, tg):
                # K projection of chunk 0 restricted to key columns [q0, q1)
                w = q1 - q0
                ps = pool.tile([P, QC], F32, tag=tg, name=f"kp0_{j}_{q0}")
                for k in range(KT):
                    nc.tensor.matmul(
                        ps[:, 0:w], wk_sb[:, k, j * P:(j + 1) * P],
                        xk_sb[:, 0, k, q0:q1],
                        start=(k == 0), stop=(k == KT - 1))
                nc.vector.tensor_tensor(
                    kt_sb[:, j, q0:q1], ps[:, 0:w],
                    bk_sb[:, j, None].to_broadcast((P, w)), ADD)

            # ---- warmup: K block 0 (128 keys), Q chunk 0, first S/exp,
            # then the rest of K chunk 0 and V block 0 ----
            k_proj0_part(0, 0, P, yp, "y")
            q_proj(0, YZ, half=0)
            k_proj0_part(1, 0, P, zp, "z")
            q_proj(0, YZ, half=1)
            ats_chunk = s_exp(0, 0)
            k_proj0_part(0, P, QC, yp, "y")
            k_proj0_part(1, P, QC, zp, "z")
            v_proj(0, YZ)

            for c in range(NCH):
                av0 = avp.tile([P, QC], F32, tag="av", name=f"av0_{c}")
                av1 = avp.tile([P, QC], F32, tag="av", name=f"av1_{c}")
                zps = zp.tile([P, QC], F32, tag="z", name=f"zps_{c}")
                avs[c] = (av0, av1)

                # S/exp runs one key-block ahead of AV so accumulator-reuse
                # waits (normalize of chunk c-1) never starve the ACT.
                ats_next = ats_chunk

                for kt in range(NKT):
                    ats = ats_next

                    if kt + 1 < NKT:
                        ats_next = s_exp(c, kt + 1)

                    # -- interleaved projection / output work on the PE;
                    # emitted after the next S/exp so it never gates ACT --
                    if c == 0:
                        if kt in (1, 2):
                            k_proj(1, YO, half=kt - 1)
                        if kt in (5, 6):
                            k_proj(2, YO, half=kt - 5)
                        if kt in (9, 10):
                            k_proj(3, YO, half=kt - 9)
                        if kt < NKT - 1:
                            v_proj(kt + 1, YO)
                    if c > 0 and 4 <= kt <= 11:
                        y_tile(c - 1, kt - 4, YO)
                    if c < NCH - 1 and 11 <= kt <= 14:
                        j, part = (kt - 11) // 2, (kt - 11) % 2
                        q_proj(c + 1, YO, half=j,
                               ks=(part * 4, 4 + part * 4))

                    if kt == 0:
                        # zero-matmuls set has_written across each whole bank
                        # so the col-tiled groups below can accumulate
                        rhs0 = xk_sb[:, 0, 0, :]
                        nc.tensor.matmul(av0[:], zeros_w[:], rhs0,
                                         start=True, stop=False)
                        nc.tensor.matmul(av1[:], zeros_w[:], rhs0,
                                         start=True, stop=False)
                        nc.tensor.matmul(zps[:], zeros_w[:], rhs0,
                                         start=True, stop=False)

                    last = kt == NKT - 1
                    if last:
                        # Z first so the denominator chain starts earlier
                        for h in range(4):
                            nc.tensor.matmul(
                                zps[32 * h:32 * h + 1, :],
                                ones_c[:],
                                ats[h // 2][:, (h % 2) * QC:(h % 2 + 1) * QC],
                                start=False, stop=(h == 3),
                                tile_position=(0, 32 * h))
                    for pair in range(2):
                        at = ats[pair]
                        av = avs[c][pair]
                        for hp in range(2):
                            h = 2 * pair + hp
                            nc.tensor.matmul(
                                av[64 * hp:64 * hp + 64, :],
                                v_sb[:, kt, h, :], at[:, hp * QC:(hp + 1) * QC],
                                start=False, stop=(last and hp == 1),
                                tile_position=(0, 64 * hp))
                    if not last:
                        for h in range(4):
                            nc.tensor.matmul(
                                zps[32 * h:32 * h + 1, :],
                                ones_c[:],
                                ats[h // 2][:, (h % 2) * QC:(h % 2 + 1) * QC],
                                start=False, stop=False,
                                tile_position=(0, 32 * h))

                # next chunk's first S/exp goes ahead of the normalize chain
                if c + 1 < NCH:
                    ats_chunk = s_exp(c + 1, 0)

                # -- free the av banks fast: unnormalized copy to f16 SBUF
                # (next chunk's zero-matmuls reuse the banks ~1.5us later,
                # instead of waiting for the whole 1/Z broadcast chain) --
                zr = rzs.tile([1, 4 * QC], F32, tag="zr", name=f"zr_{c}")
                for h in range(4):
                    nc.vector.tensor_copy(zr[0:1, h * QC:(h + 1) * QC],
                                          zps[32 * h:32 * h + 1, :])
                oun = osb.tile([P, 2, QC], F16, tag="oun", name=f"oun_{c}")
                for pair in range(2):
                    nc.vector.tensor_copy(oun[:, pair, :], avs[c][pair][:])

                # -- softmax denominators: stage via DRAM to broadcast --
                z_dram = z_st[c:c + 1, :]
                nc.sync.dma_start(z_dram, zr[:])
                z_v = z_dram.rearrange("a (h q) -> (a h) q", h=4)
                o_sb = osb.tile([P, 2, QC], BF16, tag="o", name=f"o_{c}")
                o_tiles[c] = o_sb
                for pair in range(2):
                    # after the last exp the scalar queue is free: use it to
                    # halve the tail's broadcast latency on the final chunk
                    eng = nc.scalar if (c == NCH - 1 and pair == 1) else nc.sync
                    zb = rzs.tile([P, QC], F32, tag="zb", name=f"zb_{c}_{pair}")
                    for hp in range(2):
                        h = 2 * pair + hp
                        eng.dma_start(
                            zb[64 * hp:64 * hp + 64, :],
                            z_v[h, None, :].to_broadcast((64, QC)))
                    rzb = rzs.tile([P, QC], F32, tag="rzb", name=f"rzb_{c}_{pair}")
                    nc.vector.reciprocal_approx_fast(rzb[:], zb[:])
                    nc.vector.tensor_tensor(
                        o_sb[:, pair, :], oun[:, pair, :], rzb[:], MULT)

            # tail: out-projection for the last chunk
            for m in range(8):
                y_tile(NCH - 1, m, YZ)

    nc.compile()
    return nc


def get_program():
    global _PROGRAM
    if _PROGRAM is None:
        _PROGRAM = _build_program()
    return _PROGRAM


BF = ml_dtypes.bfloat16


def _tile_xT(x, nchunk, width):
    # x [n, 1024] -> x^T tiled [128 p, nchunk, 8 k, width] bf16
    xt = np.ascontiguousarray(x.T)                      # [1024, n]
    return np.ascontiguousarray(
        xt.reshape(KT, P, nchunk, width).transpose(1, 2, 0, 3).astype(BF))


def _tile_w(w_rows):
    # w_rows [256, 1024] (= W[g-slice]) -> W^T tiled [128 p, 8 k, 256] bf16
    return np.ascontiguousarray(
        w_rows.T.reshape(KT, P, DG).transpose(1, 0, 2).astype(BF))


def make_in_maps(queries, keys, values, Wq, bq, Wk, bk, Wv, bv, Wo, bo):
    """Build per-core input dicts. Core c handles batch c//4, head group c%4."""
    f32 = np.float32
    xT = {}
    for ib in range(B):
        xT[ib] = (
            _tile_xT(np.asarray(queries[ib], f32), NCH, QC),
            _tile_xT(np.asarray(keys[ib], f32), NCH, QC),
            _tile_xT(np.asarray(values[ib], f32), NKT, P),
        )
    ones_c = np.ones((P, 1), BF)
    zeros_w = np.zeros((P, P), BF)
    in_maps = []
    for core in range(8):
        ib, g = core // G, core % G
        sl = slice(g * DG, (g + 1) * DG)
        in_maps.append({
            "xqT": xT[ib][0], "xkT": xT[ib][1], "xvT": xT[ib][2],
            "wqT": _tile_w(Wq[sl, :]),
            "wkT": _tile_w(Wk[sl, :]),
            "wvT": _tile_w(Wv[sl, :]),
            "woT": np.ascontiguousarray(
                Wo[:, sl].T.reshape(2, P, D_MODEL).transpose(1, 0, 2).astype(BF)),
            "bq_s": np.ascontiguousarray(np.asarray(bq[sl], f32).reshape(2, P).T),
            "bk_s": np.ascontiguousarray(np.asarray(bk[sl], f32).reshape(2, P).T),
            "ones_c": ones_c,
            "zeros_w": zeros_w,
        })
    return in_maps


def gather_output(results, bias):
    out = np.zeros((B, N, D_MODEL), np.float32)
    for core in range(8):
        out[core // G] += np.asarray(results[core]["yT"], np.float32).T
    out += bias[None, None, :]
    return out


def _run(inputs, trace=False, **spmd_kwargs):
    nc = get_program()
    in_maps = make_in_maps(**inputs)
    res = run_bass_kernel_spmd(nc, in_maps, core_ids=list(range(8)),
                               trace=trace, **spmd_kwargs)
    # bv commutes through the softmax (rows sum to 1) and the linear
    # out-projection, so it lands on the host as Wo @ bv; bo is host-side.
    bias = (np.asarray(inputs["Wo"], np.float64) @ np.asarray(inputs["bv"], np.float64)
            + np.asarray(inputs["bo"], np.float64)).astype(np.float32)
    return gather_output(res.results, bias), res


def kernel(**inputs) -> np.ndarray:
    out, _ = _run(inputs, trace=False)
    return out
